# revision 14
# baseline (speedup 1.0000x reference)
"""Causal multi-head attention on 8 Trainium2 NeuronCores.

Problem: x[4,2048,1024], Wqkv[1024,3072] (H=16 heads, hd=64), causal mask,
softmax, Wproj[1024,1024] + bproj.

Sharding: (batch x head-group) across 8 cores. Core c handles batch b=c//2
and heads hg*8..hg*8+7 (hg=c%2). Each core computes QKV for its 512 head
columns, full causal attention for its 8 heads, and a partial output
projection over its 512 rows of Wproj. Host sums the two partials per batch
and adds the bias.

Per-core pipeline (all matmuls in float32r: TF32-like, 1 cyc/row, ~1.5e-4):
  xT [1024,2048] = x[b].T  (host-transposed), Q^T/K^T [512,2048] e-major,
  Vaug 16 tiles [128, 8*65] = per-head [V_h | ones] columns.
  Attention transposed: S^T[k,q] = K_blk^T.T @ Q^T (k-blocks 128, q-chunks
  512, causal block-skipped; softmax scale pre-folded into Wq). exp on
  ScalarE -> P^T. One fused matmul per block: [V|1].T @ P^T accumulates
  O^T (rows 0..63) AND the denominator (row 64) in one pass. Normalize:
  denom row -> DMA to partition 0 -> approx-reciprocal -> gpsimd
  partition_broadcast -> TT-multiply into per-head-pair O tiles (even head
  direct, odd head via cross-partition DMA repack), giving K=128 lhsT chunks
  for the projection.
"""
import numpy as np

B, S, D, H = 4, 2048, 1024, 16
HD = D // H          # 64
HPC = H // 2         # 8 heads per core
SCALE = HD ** -0.5
NCORES = 8
SBK = S // 128       # 16 s-blocks
NQ = S // 512        # 4 q-chunks
KC = D // 128        # 8 d-chunks

_cache = {}


def _build():
    import concourse.mybir as mybir
    import concourse.tile as tile
    from concourse import bacc

    F32 = mybir.dt.float32
    F32R = mybir.dt.float32r
    Exp = mybir.ActivationFunctionType.Exp
    mult = mybir.AluOpType.mult

    nc = bacc.Bacc(None, target_bir_lowering=False)
    xT = nc.dram_tensor("xT", [D, S], F32, kind="ExternalInput")
    wq = nc.dram_tensor("wq", [D, 512], F32, kind="ExternalInput")
    wk = nc.dram_tensor("wk", [D, 512], F32, kind="ExternalInput")
    wv = nc.dram_tensor("wv", [D, 512], F32, kind="ExternalInput")
    wp = nc.dram_tensor("wp", [512, D], F32, kind="ExternalInput")
    tri = nc.dram_tensor("tri", [128, 128], F32, kind="ExternalInput")
    out = nc.dram_tensor("out", [S, D], F32, kind="ExternalOutput")

    with tile.TileContext(nc) as tc:
        with tc.tile_pool(name="pers", bufs=1) as pers, \
             tc.tile_pool(name="pab", bufs=1) as pab:
            tri_r = pers.tile([128, 128], F32R, name="tri_r")
            nc.gpsimd.dma_start(tri_r[:], tri[:])
            ones_f = pers.tile([128, 8], F32, name="ones_f")
            nc.vector.memset(ones_f[:], 1.0)

            qt = [pab.tile([128, S], F32R, name=f"qt{m}") for m in range(4)]
            kt = [pab.tile([128, S], F32R, name=f"kt{m}") for m in range(4)]
            # [V_h | ones] per head, 65 columns per head
            vaug = [pab.tile([128, HPC * 65], F32R, name=f"v{i}")
                    for i in range(SBK)]

            # ---------------- Phase A: QKV projection ----------------
            with tc.tile_pool(name="xtp", bufs=1) as xtp, \
                 tc.tile_pool(name="wpa", bufs=1) as wpa, \
                 tc.tile_pool(name="psA", bufs=4, space="PSUM") as psA:
                xt = [xtp.tile([128, S], F32R, name=f"xt{k}") for k in range(KC)]

                ncopy = 0

                def psum_out(dst, ps):
                    nonlocal ncopy
                    if ncopy % 3 != 2:
                        nc.vector.tensor_copy(dst, ps)
                    else:
                        nc.scalar.copy(dst, ps)
                    ncopy += 1

                for wdram, dst in ((wq, qt), (wk, kt)):
                    wt = [wpa.tile([128, 512], F32R, name=f"w{k}_{dst[0].tensor.name}",
                                   tag=f"w{k}") for k in range(KC)]
                    for k in range(KC):
                        nc.gpsimd.dma_start(wt[k][:], wdram[k * 128:(k + 1) * 128, :])
                        if wdram is wq:
                            nc.gpsimd.dma_start(xt[k][:],
                                                xT[k * 128:(k + 1) * 128, :])
                    for m in range(4):
                        for n in range(NQ):
                            ps = psA.tile([128, 512], F32, name="psa", tag="psa")
                            for k in range(KC):
                                nc.tensor.matmul(
                                    ps[:], wt[k][:, m * 128:(m + 1) * 128],
                                    xt[k][:, n * 512:(n + 1) * 512],
                                    start=(k == 0), stop=(k == KC - 1))
                            psum_out(dst[m][:, n * 512:(n + 1) * 512], ps[:])
                wvt = [wpa.tile([128, 512], F32R, name=f"wv{k}", tag=f"w{k}")
                       for k in range(KC)]
                for k in range(KC):
                    nc.gpsimd.dma_start(wvt[k][:], wv[k * 128:(k + 1) * 128, :])
                for i in range(SBK):
                    ps = psA.tile([128, 512], F32, name="psa", tag="psa")
                    for k in range(KC):
                        nc.tensor.matmul(
                            ps[:], xt[k][:, i * 128:(i + 1) * 128], wvt[k][:],
                            start=(k == 0), stop=(k == KC - 1))
                    va3 = vaug[i].rearrange("p (h c) -> p h c", c=65)
                    psum_out(va3[:, :, 0:64],
                             ps.rearrange("p (h c) -> p h c", c=64))
                    nc.vector.tensor_copy(va3[:, :, 64:65],
                                          ones_f.rearrange("p (h c) -> p h c", c=1))

            # pools that outlive pab (B and C), placed on the right stack
            othp_cm = tc.tile_pool(name="othp", bufs=1, side="right")
            othp = othp_cm.__enter__()
            # packed per-head-pair O tiles: rows 0:64 even head, 64:128 odd
            otp2 = [othp.tile([128, S], F32R, name=f"otp{m}") for m in range(4)]
            wpt = [othp.tile([128, D], F32R, name=f"wpp{m}") for m in range(4)]
            for m in range(4):
                nc.gpsimd.dma_start(wpt[m][:], wp[m * 128:(m + 1) * 128, :])

            # ---------------- Phase B: causal attention ----------------
            with tc.tile_pool(name="ptp", bufs=2) as ptp, \
                 tc.tile_pool(name="rbp", bufs=2) as rbp, \
                 tc.tile_pool(name="psS", bufs=2, space="PSUM") as psS, \
                 tc.tile_pool(name="psO", bufs=1, space="PSUM") as psO:
                for h in range(HPC):
                    mt, pr = h // 2, (h % 2) * 64
                    odd = h % 2 == 1
                    if odd:
                        otx = rbp.tile([64, S], F32R, name="otx", tag="otx",
                                       bufs=2)
                    oavds = []
                    for J in range(NQ):
                        nblk = 4 * J + 4
                        qs = slice(J * 512, (J + 1) * 512)
                        oavd = psO.tile([65, 512], F32, name="oavd",
                                        tag=f"oavd{J}")
                        oavds.append(oavd)
                        for g0 in range(0, nblk, 2):
                            grp = list(range(g0, min(g0 + 2, nblk)))
                            stg = psS.tile([128, 1024], F32, name="stg", tag="stg")
                            for gi, i in enumerate(grp):
                                nc.tensor.matmul(
                                    stg[:, gi * 512:(gi + 1) * 512],
                                    kt[mt][pr:pr + 64, i * 128:(i + 1) * 128],
                                    qt[mt][pr:pr + 64, qs],
                                    start=True, stop=True)
                            pt = ptp.tile([128, 1024], F32R, name="pt", tag="pt")
                            wg = len(grp) * 512
                            nc.scalar.activation(pt[:, :wg], stg[:, :wg], Exp)
                            for gi, i in enumerate(grp):
                                if i >= 4 * J:  # diagonal block: mask triangle
                                    w0 = 128 * i - 512 * J
                                    sl = pt[:, gi * 512 + w0: gi * 512 + w0 + 128]
                                    nc.vector.tensor_tensor(sl, sl, tri_r[:], op=mult)
                            for gi, i in enumerate(grp):
                                w0 = max(0, 128 * i - 512 * J)
                                nc.tensor.matmul(
                                    oavd[:, w0:],
                                    vaug[i][:, h * 65:(h + 1) * 65],
                                    pt[:, gi * 512 + w0:(gi + 1) * 512],
                                    start=(i == 0), stop=(i == nblk - 1))
                    # batched normalize for the whole head: denom rows ->
                    # partition 0 -> recip -> broadcast -> TT per chunk
                    d1 = rbp.tile([65, S], F32, name="d1", tag="d1", bufs=1)
                    for J in range(NQ):
                        nc.scalar.copy(d1[64:65, J * 512:(J + 1) * 512],
                                       oavds[J][64:65, :])
                    d0 = rbp.tile([1, S], F32, name="d0", tag="d0", bufs=1)
                    nc.gpsimd.dma_start(d0[:], d1[64:65, :])
                    nc.vector.reciprocal_approx_fast(d0[:], d0[:])
                    rb = rbp.tile([64, S], F32, name="rb", tag="rb")
                    nc.gpsimd.partition_broadcast(rb[:], d0[:])
                    for J in range(NQ):
                        qs = slice(J * 512, (J + 1) * 512)
                        dst = otx[:, qs] if odd else otp2[mt][0:64, qs]
                        nc.vector.tensor_tensor(dst, oavds[J][0:64, :],
                                                rb[:, qs], op=mult)
                    if odd:
                        nc.sync.dma_start(otp2[mt][64:128, :], otx[:])

        # ---------------- Phase C: output projection ----------------
        with tc.tile_pool(name="psC", bufs=2, space="PSUM") as psC, \
             tc.tile_pool(name="obp", bufs=3) as obp:
            for s in range(SBK):
                pp = psC.tile([128, 1024], F32, name="pp", tag="pp")
                for n2 in range(2):
                    for m in range(4):
                        nc.tensor.matmul(
                            pp[:, n2 * 512:(n2 + 1) * 512],
                            otp2[m][:, s * 128:(s + 1) * 128],
                            wpt[m][:, n2 * 512:(n2 + 1) * 512],
                            start=(m == 0), stop=(m == 3))
                ob = obp.tile([128, 1024], F32, name="ob", tag="ob")
                nc.vector.tensor_copy(ob[:, 0:512], pp[:, 0:512])
                nc.scalar.copy(ob[:, 512:1024], pp[:, 512:1024])
                nc.sync.dma_start(out[s * 128:(s + 1) * 128, :], ob[:])
        othp_cm.__exit__(None, None, None)
    nc.finalize()
    return nc


def _get_nc():
    if "nc" not in _cache:
        _cache["nc"] = _build()
    return _cache["nc"]


def kernel(x, mask, Wqkv, Wproj, bproj):
    from concourse.bass_utils import run_bass_kernel_spmd

    x = np.asarray(x, dtype=np.float32)
    Wqkv = np.asarray(Wqkv, dtype=np.float32)
    Wproj = np.asarray(Wproj, dtype=np.float32)
    bproj = np.asarray(bproj, dtype=np.float32)

    tri = np.ascontiguousarray(np.triu(np.ones((128, 128), dtype=np.float32)))
    xTs = [np.ascontiguousarray(x[b].T) for b in range(B)]
    in_maps = []
    for c in range(NCORES):
        b, hg = c // 2, c % 2
        cs = slice(hg * 512, (hg + 1) * 512)
        in_maps.append(dict(
            xT=xTs[b],
            wq=np.ascontiguousarray(Wqkv[:, 0 * D:1 * D][:, cs] * SCALE),
            wk=np.ascontiguousarray(Wqkv[:, 1 * D:2 * D][:, cs]),
            wv=np.ascontiguousarray(Wqkv[:, 2 * D:3 * D][:, cs]),
            wp=np.ascontiguousarray(Wproj[cs, :]),
            tri=tri,
        ))

    res = run_bass_kernel_spmd(_get_nc(), in_maps, core_ids=list(range(NCORES)),
                               **_cache.get("run_kwargs", {}))
    _cache["last_result"] = res
    parts = [r["out"] for r in res.results]
    outp = np.stack([parts[2 * b] + parts[2 * b + 1] for b in range(B)])
    outp += bproj
    return outp.astype(np.float32)


# revision 15
# speedup vs baseline: 1.1456x; 1.1456x over previous
"""Causal multi-head attention on 8 Trainium2 NeuronCores.

Problem: x[4,2048,1024], Wqkv[1024,3072] (H=16 heads, hd=64), causal mask,
softmax, Wproj[1024,1024] + bproj.

Sharding: (batch x head-group) across 8 cores. Core c handles batch b=c//2
and heads hg*8..hg*8+7 (hg=c%2). Each core computes QKV for its 512 head
columns, full causal attention for its 8 heads, and a partial output
projection over its 512 rows of Wproj. Host sums the two partials per batch
and adds the bias.

Per-core pipeline (all matmuls in float32r: TF32-like, 1 cyc/row, ~1.5e-4):
  xT [1024,2048] = x[b].T  (host-transposed), Q^T/K^T [512,2048] e-major,
  Vaug 16 tiles [128, 8*65] = per-head [V_h | ones] columns.
  Attention transposed: S^T[k,q] = K_blk^T.T @ Q^T (k-blocks 128, q-chunks
  512, causal block-skipped; softmax scale pre-folded into Wq). exp on
  ScalarE -> P^T. One fused matmul per block: [V|1].T @ P^T accumulates
  O^T (rows 0..63) AND the denominator (row 64) in one pass. Normalize:
  denom row -> DMA to partition 0 -> approx-reciprocal -> gpsimd
  partition_broadcast -> TT-multiply into per-head-pair O tiles (even head
  direct, odd head via cross-partition DMA repack), giving K=128 lhsT chunks
  for the projection.
"""
import numpy as np

B, S, D, H = 4, 2048, 1024, 16
HD = D // H          # 64
HPC = H // 2         # 8 heads per core
SCALE = HD ** -0.5
NCORES = 8
SBK = S // 128       # 16 s-blocks
NQ = S // 512        # 4 q-chunks
KC = D // 128        # 8 d-chunks

_cache = {}


def _build():
    import concourse.mybir as mybir
    import concourse.tile as tile
    from concourse import bacc

    F32 = mybir.dt.float32
    F32R = mybir.dt.float32r
    Exp = mybir.ActivationFunctionType.Exp
    mult = mybir.AluOpType.mult

    nc = bacc.Bacc(None, target_bir_lowering=False)
    xT = nc.dram_tensor("xT", [D, S], F32, kind="ExternalInput")
    wq = nc.dram_tensor("wq", [D, 512], F32, kind="ExternalInput")
    wk = nc.dram_tensor("wk", [D, 512], F32, kind="ExternalInput")
    wv = nc.dram_tensor("wv", [D, 512], F32, kind="ExternalInput")
    wp = nc.dram_tensor("wp", [512, D], F32, kind="ExternalInput")
    tri = nc.dram_tensor("tri", [128, 128], F32, kind="ExternalInput")
    out = nc.dram_tensor("out", [S, D], F32, kind="ExternalOutput")

    with tile.TileContext(nc) as tc:
        with tc.tile_pool(name="pers", bufs=1) as pers, \
             tc.tile_pool(name="pab", bufs=1) as pab:
            tri_r = pers.tile([128, 128], F32R, name="tri_r")
            nc.gpsimd.dma_start(tri_r[:], tri[:])
            ones_f = pers.tile([128, 8], F32, name="ones_f")
            nc.vector.memset(ones_f[:], 1.0)

            qt = [pab.tile([128, S], F32R, name=f"qt{m}") for m in range(4)]
            kt = [pab.tile([128, S], F32R, name=f"kt{m}") for m in range(4)]
            # [V_h | ones] per head, 65 columns per head
            vaug = [pab.tile([128, HPC * 65], F32R, name=f"v{i}")
                    for i in range(SBK)]

            # ---------------- Phase A: QKV projection ----------------
            with tc.tile_pool(name="xtp", bufs=1) as xtp, \
                 tc.tile_pool(name="wpa", bufs=1) as wpa, \
                 tc.tile_pool(name="psA", bufs=4, space="PSUM") as psA:
                xt = [xtp.tile([128, S], F32R, name=f"xt{k}") for k in range(KC)]

                ncopy = 0

                def psum_out(dst, ps):
                    nonlocal ncopy
                    if ncopy % 3 != 2:
                        nc.vector.tensor_copy(dst, ps)
                    else:
                        nc.scalar.copy(dst, ps)
                    ncopy += 1

                for wdram, dst in ((wq, qt), (wk, kt)):
                    wt = [wpa.tile([128, 512], F32R, name=f"w{k}_{dst[0].tensor.name}",
                                   tag=f"w{k}") for k in range(KC)]
                    for k in range(KC):
                        nc.gpsimd.dma_start(wt[k][:], wdram[k * 128:(k + 1) * 128, :])
                        if wdram is wq:
                            nc.gpsimd.dma_start(xt[k][:],
                                                xT[k * 128:(k + 1) * 128, :])
                    for m in range(4):
                        for n in range(NQ):
                            ps = psA.tile([128, 512], F32, name="psa", tag="psa")
                            for k in range(KC):
                                nc.tensor.matmul(
                                    ps[:], wt[k][:, m * 128:(m + 1) * 128],
                                    xt[k][:, n * 512:(n + 1) * 512],
                                    start=(k == 0), stop=(k == KC - 1))
                            psum_out(dst[m][:, n * 512:(n + 1) * 512], ps[:])
                wvt = [wpa.tile([128, 512], F32R, name=f"wv{k}", tag=f"w{k}")
                       for k in range(KC)]
                for k in range(KC):
                    nc.gpsimd.dma_start(wvt[k][:], wv[k * 128:(k + 1) * 128, :])
                for i in range(SBK):
                    ps = psA.tile([128, 512], F32, name="psa", tag="psa")
                    for k in range(KC):
                        nc.tensor.matmul(
                            ps[:], xt[k][:, i * 128:(i + 1) * 128], wvt[k][:],
                            start=(k == 0), stop=(k == KC - 1))
                    va3 = vaug[i].rearrange("p (h c) -> p h c", c=65)
                    psum_out(va3[:, :, 0:64],
                             ps.rearrange("p (h c) -> p h c", c=64))
                    nc.vector.tensor_copy(va3[:, :, 64:65],
                                          ones_f.rearrange("p (h c) -> p h c", c=1))

            # pools that outlive pab (B and C), placed on the right stack
            othp_cm = tc.tile_pool(name="othp", bufs=1, side="right")
            othp = othp_cm.__enter__()
            # packed per-head-pair O tiles: rows 0:64 even head, 64:128 odd
            otp2 = [othp.tile([128, S], F32R, name=f"otp{m}") for m in range(4)]
            wpt = [othp.tile([128, D], F32R, name=f"wpp{m}") for m in range(4)]
            for m in range(4):
                nc.gpsimd.dma_start(wpt[m][:], wp[m * 128:(m + 1) * 128, :])

            # ---------------- Phase B: causal attention ----------------
            with tc.tile_pool(name="ptp", bufs=2) as ptp, \
                 tc.tile_pool(name="rbp", bufs=2) as rbp, \
                 tc.tile_pool(name="psS", bufs=3, space="PSUM") as psS, \
                 tc.tile_pool(name="psO", bufs=2, space="PSUM") as psO:
                for h in range(HPC):
                    mt, pr = h // 2, (h % 2) * 64
                    odd = h % 2 == 1
                    if odd:
                        otx = rbp.tile([64, S], F32R, name="otx", tag="otx",
                                       bufs=2)
                    for J in range(NQ):
                        nblk = 4 * J + 4
                        qs = slice(J * 512, (J + 1) * 512)
                        oavd = psO.tile([65, 512], F32, name="oavd", tag="oavd")
                        for g0 in range(0, nblk, 2):
                            grp = list(range(g0, min(g0 + 2, nblk)))
                            stg = psS.tile([128, 1024], F32, name="stg", tag="stg")
                            for gi, i in enumerate(grp):
                                nc.tensor.matmul(
                                    stg[:, gi * 512:(gi + 1) * 512],
                                    kt[mt][pr:pr + 64, i * 128:(i + 1) * 128],
                                    qt[mt][pr:pr + 64, qs],
                                    start=True, stop=True)
                            pt = ptp.tile([128, 1024], F32R, name="pt", tag="pt")
                            wg = len(grp) * 512
                            nc.scalar.activation(pt[:, :wg], stg[:, :wg], Exp)
                            for gi, i in enumerate(grp):
                                if i >= 4 * J:  # diagonal block: mask triangle
                                    w0 = 128 * i - 512 * J
                                    sl = pt[:, gi * 512 + w0: gi * 512 + w0 + 128]
                                    nc.vector.tensor_tensor(sl, sl, tri_r[:], op=mult)
                            for gi, i in enumerate(grp):
                                w0 = max(0, 128 * i - 512 * J)
                                nc.tensor.matmul(
                                    oavd[:, w0:],
                                    vaug[i][:, h * 65:(h + 1) * 65],
                                    pt[:, gi * 512 + w0:(gi + 1) * 512],
                                    start=(i == 0), stop=(i == nblk - 1))
                        # one copy frees the psum bank; normalize runs in SBUF
                        osb = rbp.tile([65, 512], F32, name="osb", tag="osb",
                                       bufs=3)
                        nc.scalar.copy(osb[:], oavd[:])
                        d0 = rbp.tile([1, 512], F32, name="d0", tag="d0")
                        nc.gpsimd.dma_start(d0[:], osb[64:65, :])
                        nc.vector.reciprocal_approx_fast(d0[:], d0[:])
                        rb = rbp.tile([64, 512], F32, name="rb", tag="rb")
                        nc.gpsimd.partition_broadcast(rb[:], d0[:])
                        dst = otx[:, qs] if odd else otp2[mt][0:64, qs]
                        nc.vector.tensor_tensor(dst, osb[0:64, :], rb[:], op=mult)
                    if odd:
                        nc.sync.dma_start(otp2[mt][64:128, :], otx[:])

        # ---------------- Phase C: output projection ----------------
        with tc.tile_pool(name="psC", bufs=2, space="PSUM") as psC, \
             tc.tile_pool(name="obp", bufs=3) as obp:
            for s in range(SBK):
                pp = psC.tile([128, 1024], F32, name="pp", tag="pp")
                for n2 in range(2):
                    for m in range(4):
                        nc.tensor.matmul(
                            pp[:, n2 * 512:(n2 + 1) * 512],
                            otp2[m][:, s * 128:(s + 1) * 128],
                            wpt[m][:, n2 * 512:(n2 + 1) * 512],
                            start=(m == 0), stop=(m == 3))
                ob = obp.tile([128, 1024], F32, name="ob", tag="ob")
                nc.vector.tensor_copy(ob[:, 0:512], pp[:, 0:512])
                nc.scalar.copy(ob[:, 512:1024], pp[:, 512:1024])
                nc.sync.dma_start(out[s * 128:(s + 1) * 128, :], ob[:])
        othp_cm.__exit__(None, None, None)
    nc.finalize()
    return nc


def _get_nc():
    if "nc" not in _cache:
        _cache["nc"] = _build()
    return _cache["nc"]


def kernel(x, mask, Wqkv, Wproj, bproj):
    from concourse.bass_utils import run_bass_kernel_spmd

    x = np.asarray(x, dtype=np.float32)
    Wqkv = np.asarray(Wqkv, dtype=np.float32)
    Wproj = np.asarray(Wproj, dtype=np.float32)
    bproj = np.asarray(bproj, dtype=np.float32)

    tri = np.ascontiguousarray(np.triu(np.ones((128, 128), dtype=np.float32)))
    xTs = [np.ascontiguousarray(x[b].T) for b in range(B)]
    in_maps = []
    for c in range(NCORES):
        b, hg = c // 2, c % 2
        cs = slice(hg * 512, (hg + 1) * 512)
        in_maps.append(dict(
            xT=xTs[b],
            wq=np.ascontiguousarray(Wqkv[:, 0 * D:1 * D][:, cs] * SCALE),
            wk=np.ascontiguousarray(Wqkv[:, 1 * D:2 * D][:, cs]),
            wv=np.ascontiguousarray(Wqkv[:, 2 * D:3 * D][:, cs]),
            wp=np.ascontiguousarray(Wproj[cs, :]),
            tri=tri,
        ))

    res = run_bass_kernel_spmd(_get_nc(), in_maps, core_ids=list(range(NCORES)),
                               **_cache.get("run_kwargs", {}))
    _cache["last_result"] = res
    parts = [r["out"] for r in res.results]
    outp = np.stack([parts[2 * b] + parts[2 * b + 1] for b in range(B)])
    outp += bproj
    return outp.astype(np.float32)
